# revision 1
# baseline (speedup 1.0000x reference)
"""Windowed attention (swin-style, 49-token windows, 8 heads) with DynamicPosBias.

Strategy: data-parallel over B=2048 windows -> 256 windows/core on 8 cores.
Host pre-transposes q,k per head to [W, 8, 64, 49] so the device needs no
transposes; v gets a fused ones-column so PV matmul also produces the softmax
denominator. Position-bias MLP runs on device once; the bias is fused into the
QK matmul as 49 extra contraction rows (lhsT=[K^T;I49], rhs=[Q^T;8*rpb]) and
exp(0.125*S) folds the 1/sqrt(64) scale.
"""

import numpy as np
from contextlib import ExitStack

import concourse.bass as bass
import concourse.mybir as mybir
import concourse.tile as tile
from concourse import bacc
from concourse.bass_utils import run_bass_kernel_spmd

G = 7
NTOK = 49          # tokens per window
H = 8              # heads
HD = 64            # head dim
C = 512
B = 2048
NCORES = 8
W = B // NCORES    # windows per core
T = (2 * G - 1) ** 2  # 169 bias table entries
PDIM = 32          # MLP hidden
NBUF = 4
F32 = mybir.dt.float32
F16 = mybir.dt.float16
I32 = mybir.dt.int32


def _rel_idx():
    coords = np.stack(np.meshgrid(np.arange(G), np.arange(G), indexing="ij")).reshape(2, -1)
    rel = (coords[:, :, None] - coords[:, None, :]).transpose(1, 2, 0)
    rel = rel.copy()
    rel[:, :, 0] += G - 1
    rel[:, :, 1] += G - 1
    rel[:, :, 0] *= 2 * G - 1
    return rel.sum(-1)  # [i, j] in [0, 169)


def _biases_t():
    pb = np.arange(1 - G, G, dtype=np.float32)
    b = np.stack(np.meshgrid(pb, pb, indexing="ij")).reshape(2, -1)  # [2, 169]
    return np.ascontiguousarray(b.astype(np.float32))


_CACHED_NC = None
LAST_RESULTS = None


def _build_nc():
    global _CACHED_NC
    if _CACHED_NC is not None:
        return _CACHED_NC
    nc = bacc.Bacc(None, target_bir_lowering=False)

    qt_d = nc.dram_tensor("qt", [W, H, HD, NTOK], F16, kind="ExternalInput")
    kt_d = nc.dram_tensor("kt", [W, H, HD, NTOK], F16, kind="ExternalInput")
    v_d = nc.dram_tensor("vaug", [W, NTOK, H * 65], F16, kind="ExternalInput")
    id8_d = nc.dram_tensor("ident8", [NTOK, H * NTOK], F16, kind="ExternalInput")
    ridx_d = nc.dram_tensor("relidx", [NTOK, NTOK], I32, kind="ExternalInput")
    bia_d = nc.dram_tensor("biases_t", [2, T], F32, kind="ExternalInput")
    ppw_d = nc.dram_tensor("pos_proj_w", [2, PDIM], F32, kind="ExternalInput")
    ppb_d = nc.dram_tensor("pos_proj_b", [PDIM], F32, kind="ExternalInput")
    mlp_vec = {}
    for nm in ["ln1_g", "ln1_b", "b1", "ln2_g", "ln2_b", "b2", "ln3_g", "ln3_b"]:
        mlp_vec[nm] = nc.dram_tensor(nm, [PDIM], F32, kind="ExternalInput")
    w1_d = nc.dram_tensor("w1", [PDIM, PDIM], F32, kind="ExternalInput")
    w2_d = nc.dram_tensor("w2", [PDIM, PDIM], F32, kind="ExternalInput")
    w3_d = nc.dram_tensor("w3", [PDIM, H], F32, kind="ExternalInput")
    b3_d = nc.dram_tensor("b3", [H], F32, kind="ExternalInput")
    out_d = nc.dram_tensor("out", [W, NTOK, C], F32, kind="ExternalOutput")
    pos_dram = nc.dram_tensor("pos_scratch", [T, H], F32, kind="Internal")

    with tile.TileContext(nc) as tc, ExitStack() as ctx:
        const = ctx.enter_context(tc.tile_pool(name="const", bufs=1))
        mlp = ctx.enter_context(tc.tile_pool(name="mlp", bufs=1))
        mps = ctx.enter_context(tc.tile_pool(name="mps", bufs=1, space="PSUM"))

        # ---------- DynamicPosBias MLP: X^T layout [feat, 169] ----------
        biasesT = mlp.tile([2, T], F32, tag="biasesT")
        nc.sync.dma_start(biasesT[:], bia_d[:])
        ppw = mlp.tile([2, PDIM], F32, tag="ppw")
        nc.sync.dma_start(ppw[:], ppw_d[:])
        vec_sb = {}
        for nm in ["ln1_g", "ln1_b", "b1", "ln2_g", "ln2_b", "b2", "ln3_g", "ln3_b"]:
            t = mlp.tile([PDIM, 1], F32, tag=nm)
            nc.sync.dma_start(t[:], mlp_vec[nm][:])
            vec_sb[nm] = t
        ppb = mlp.tile([PDIM, 1], F32, tag="ppb")
        nc.sync.dma_start(ppb[:], ppb_d[:])
        w1 = mlp.tile([PDIM, PDIM], F32, tag="w1")
        nc.sync.dma_start(w1[:], w1_d[:])
        w2 = mlp.tile([PDIM, PDIM], F32, tag="w2")
        nc.sync.dma_start(w2[:], w2_d[:])
        w3 = mlp.tile([PDIM, H], F32, tag="w3")
        nc.sync.dma_start(w3[:], w3_d[:])
        b3 = mlp.tile([H, 1], F32, tag="b3")
        nc.sync.dma_start(b3[:], b3_d[:])

        eps_t = mlp.tile([1, 1], F32, tag="eps")
        nc.gpsimd.memset(eps_t[:], 1e-5)
        ones_inv = mlp.tile([PDIM, 1], F32, tag="ones_inv")
        nc.gpsimd.memset(ones_inv[:], 1.0 / PDIM)
        ones_row = mlp.tile([1, PDIM], F32, tag="ones_row")
        nc.gpsimd.memset(ones_row[:], 1.0)

        x_ps = mps.tile([PDIM, T], F32, tag="mpsA")
        nc.tensor.matmul(out=x_ps[:], lhsT=ppw[:], rhs=biasesT[:], start=True, stop=True)
        x_sb = mlp.tile([PDIM, T], F32, tag="x_sb")
        nc.vector.tensor_scalar_add(out=x_sb[:], in0=x_ps[:], scalar1=ppb[:])

        layer_params = [
            (vec_sb["ln1_g"], vec_sb["ln1_b"], w1, vec_sb["b1"], PDIM),
            (vec_sb["ln2_g"], vec_sb["ln2_b"], w2, vec_sb["b2"], PDIM),
            (vec_sb["ln3_g"], vec_sb["ln3_b"], w3, b3, H),
        ]
        for li, (g_ap, bln_ap, w_ap, bout_ap, odim) in enumerate(layer_params):
            mu_ps = mps.tile([1, T], F32, tag="mpsA")
            nc.tensor.matmul(out=mu_ps[:], lhsT=ones_inv[:], rhs=x_sb[:], start=True, stop=True)
            mu_sb = mlp.tile([1, T], F32, tag=f"mus{li}")
            nc.vector.tensor_copy(mu_sb[:], mu_ps[:])
            mub_ps = mps.tile([PDIM, T], F32, tag="mpsA")
            nc.tensor.matmul(out=mub_ps[:], lhsT=ones_row[:], rhs=mu_sb[:], start=True, stop=True)
            xc = mlp.tile([PDIM, T], F32, tag=f"xc{li}")
            nc.vector.tensor_tensor(out=xc[:], in0=x_sb[:], in1=mub_ps[:], op=mybir.AluOpType.subtract)
            sq = mlp.tile([PDIM, T], F32, tag=f"sq{li}")
            nc.vector.tensor_tensor(out=sq[:], in0=xc[:], in1=xc[:], op=mybir.AluOpType.mult)
            var_ps = mps.tile([1, T], F32, tag="mpsA")
            nc.tensor.matmul(out=var_ps[:], lhsT=ones_inv[:], rhs=sq[:], start=True, stop=True)
            sd = mlp.tile([1, T], F32, tag=f"sd{li}")
            nc.scalar.activation(sd[:], var_ps[:], mybir.ActivationFunctionType.Sqrt, bias=eps_t[:])
            rstd = mlp.tile([1, T], F32, tag=f"rstd{li}")
            nc.vector.reciprocal(rstd[:], sd[:])
            rstdb_ps = mps.tile([PDIM, T], F32, tag="mpsA")
            nc.tensor.matmul(out=rstdb_ps[:], lhsT=ones_row[:], rhs=rstd[:], start=True, stop=True)
            xh = mlp.tile([PDIM, T], F32, tag=f"xh{li}")
            nc.vector.tensor_tensor(out=xh[:], in0=xc[:], in1=rstdb_ps[:], op=mybir.AluOpType.mult)
            hrelu = mlp.tile([PDIM, T], F32, tag=f"hr{li}")
            nc.scalar.activation(hrelu[:], xh[:], mybir.ActivationFunctionType.Relu,
                                 bias=bln_ap[:], scale=g_ap[:])
            xn_ps = mps.tile([odim, T], F32, tag="mpsA")
            nc.tensor.matmul(out=xn_ps[:], lhsT=w_ap[:], rhs=hrelu[:], start=True, stop=True)
            x_sb = mlp.tile([odim, T], F32, tag=f"xsb{li}")
            nc.vector.tensor_scalar_add(out=x_sb[:], in0=xn_ps[:], scalar1=bout_ap[:])

        # x_sb is now pos^T [8, 169]; push to DRAM as [169, 8] (slow tiny DMA)
        nc.sync.dma_start(pos_dram[:].rearrange("t (h o) -> h t o", o=1), x_sb[:])

        # ---------- gather rpb: 49 row-gathers -> [49, (i,h)] then reorder ----------
        ridx_sb = const.tile([NTOK, NTOK], I32, tag="ridx")
        nc.sync.dma_start(ridx_sb[:], ridx_d[:])
        rpb_tmp = const.tile([NTOK, NTOK * H], F32, tag="rpb_tmp")
        for i in range(NTOK):
            nc.gpsimd.indirect_dma_start(
                out=rpb_tmp[:, H * i : H * i + H],
                out_offset=None,
                in_=pos_dram[:],
                in_offset=bass.IndirectOffsetOnAxis(ap=ridx_sb[:, i : i + 1], axis=0),
            )
        rpb_sb = const.tile([NTOK, H * NTOK], F16, tag="rpb_sb")
        nc.vector.tensor_scalar_mul(
            out=rpb_sb[:].rearrange("p (h i) -> p h i", h=H),
            in0=rpb_tmp[:].rearrange("p (i h) -> p h i", h=H),
            scalar1=8.0,
        )

        # ---------- persistent per-slot QT/KT buffers ----------
        qt_slots = [const.tile([128, H * NTOK], F16, tag=f"qts{s}", name=f"qts{s}") for s in range(NBUF)]
        kt_slots = [const.tile([128, H * NTOK], F16, tag=f"kts{s}", name=f"kts{s}") for s in range(NBUF)]
        for s in range(NBUF):
            nc.sync.dma_start(qt_slots[s][HD : HD + NTOK, :], rpb_sb[:, :])
            nc.sync.dma_start(kt_slots[s][HD : HD + NTOK, :], id8_d[:])

        vpool = ctx.enter_context(tc.tile_pool(name="vpool", bufs=NBUF))
        epool = ctx.enter_context(tc.tile_pool(name="epool", bufs=3))
        opool = ctx.enter_context(tc.tile_pool(name="opool", bufs=3))
        rpool = ctx.enter_context(tc.tile_pool(name="rpool", bufs=3))
        stps = ctx.enter_context(tc.tile_pool(name="stps", bufs=2, space="PSUM"))
        pvps = ctx.enter_context(tc.tile_pool(name="pvps", bufs=2, space="PSUM"))

        for w in range(W):
            s = w % NBUF
            nc.sync.dma_start(
                qt_slots[s][0:HD, :].rearrange("d (h i) -> d h i", h=H),
                qt_d[w].rearrange("h d i -> d h i"),
            )
            nc.sync.dma_start(
                kt_slots[s][0:HD, :].rearrange("d (h i) -> d h i", h=H),
                kt_d[w].rearrange("h d i -> d h i"),
            )
            v_t = vpool.tile([NTOK, H * 65], F16, tag="v")
            nc.sync.dma_start(v_t[:], v_d[w])

            st = stps.tile([NTOK, H * NTOK], F32, tag="st")
            for h in range(H):
                nc.tensor.matmul(
                    out=st[:, NTOK * h : NTOK * (h + 1)],
                    lhsT=kt_slots[s][0 : HD + NTOK, NTOK * h : NTOK * (h + 1)],
                    rhs=qt_slots[s][0 : HD + NTOK, NTOK * h : NTOK * (h + 1)],
                    start=True,
                    stop=True,
                )
            ex = epool.tile([NTOK, H * NTOK], F16, tag="ex")
            nc.scalar.activation(ex[:], st[:], mybir.ActivationFunctionType.Exp, scale=0.125)

            pv0 = pvps.tile([NTOK, 4 * 65], F32, tag="pv0")
            pv1 = pvps.tile([NTOK, 4 * 65], F32, tag="pv1")
            for h in range(H):
                dst = pv0 if h < 4 else pv1
                m = h % 4
                nc.tensor.matmul(
                    out=dst[:, 65 * m : 65 * (m + 1)],
                    lhsT=ex[:, NTOK * h : NTOK * (h + 1)],
                    rhs=v_t[:, 65 * h : 65 * (h + 1)],
                    start=True,
                    stop=True,
                )
            rec = rpool.tile([NTOK, H], F32, tag="rec")
            nc.vector.reciprocal(
                rec[:, 0:4].rearrange("p (h o) -> p h o", o=1),
                pv0[:].rearrange("p (h c) -> p h c", c=65)[:, :, 64:65],
            )
            nc.vector.reciprocal(
                rec[:, 4:8].rearrange("p (h o) -> p h o", o=1),
                pv1[:].rearrange("p (h c) -> p h c", c=65)[:, :, 64:65],
            )
            o_t = opool.tile([NTOK, C], F32, tag="o")
            for half, pv in ((0, pv0), (1, pv1)):
                nc.vector.tensor_tensor(
                    out=o_t[:, 256 * half : 256 * (half + 1)].rearrange(
                        "p (h c) -> p h c", c=HD
                    ),
                    in0=pv[:].rearrange("p (h c) -> p h c", c=65)[:, :, 0:HD],
                    in1=rec[:, 4 * half : 4 * half + 4]
                    .rearrange("p (h o) -> p h o", o=1)
                    .to_broadcast([NTOK, 4, HD]),
                    op=mybir.AluOpType.mult,
                )
            nc.sync.dma_start(out_d[w], o_t[:])

    nc.finalize()
    _CACHED_NC = nc
    return nc


def kernel(q, k, v, pos_proj_w, pos_proj_b, ln1_g, ln1_b, w1, b1,
           ln2_g, ln2_b, w2, b2, ln3_g, ln3_b, w3, b3):
    q = np.ascontiguousarray(np.asarray(q, dtype=np.float32))
    k = np.ascontiguousarray(np.asarray(k, dtype=np.float32))
    v = np.ascontiguousarray(np.asarray(v, dtype=np.float32))

    ident8 = np.tile(np.eye(NTOK, dtype=np.float16), (1, H))
    relidx = np.ascontiguousarray(_rel_idx().T.astype(np.int32))  # [j, i]
    biases_t = _biases_t()

    shared = {
        "ident8": ident8, "relidx": relidx, "biases_t": biases_t,
        "pos_proj_w": np.asarray(pos_proj_w, np.float32),
        "pos_proj_b": np.asarray(pos_proj_b, np.float32),
        "ln1_g": np.asarray(ln1_g, np.float32), "ln1_b": np.asarray(ln1_b, np.float32),
        "w1": np.asarray(w1, np.float32), "b1": np.asarray(b1, np.float32),
        "ln2_g": np.asarray(ln2_g, np.float32), "ln2_b": np.asarray(ln2_b, np.float32),
        "w2": np.asarray(w2, np.float32), "b2": np.asarray(b2, np.float32),
        "ln3_g": np.asarray(ln3_g, np.float32), "ln3_b": np.asarray(ln3_b, np.float32),
        "w3": np.asarray(w3, np.float32), "b3": np.asarray(b3, np.float32),
    }

    ones_col = np.ones((W, NTOK, H, 1), dtype=np.float32)
    in_maps = []
    for c in range(NCORES):
        sl = slice(c * W, (c + 1) * W)
        qt = np.ascontiguousarray(q[sl].reshape(W, NTOK, H, HD).transpose(0, 2, 3, 1).astype(np.float16))
        kt = np.ascontiguousarray(k[sl].reshape(W, NTOK, H, HD).transpose(0, 2, 3, 1).astype(np.float16))
        vaug = np.concatenate(
            [v[sl].reshape(W, NTOK, H, HD), ones_col], axis=3
        ).reshape(W, NTOK, H * 65).astype(np.float16)
        m = dict(shared)
        m.update({"qt": qt, "kt": kt, "vaug": np.ascontiguousarray(vaug)})
        in_maps.append(m)

    nc = _build_nc()
    res = run_bass_kernel_spmd(nc, in_maps, core_ids=list(range(NCORES)))
    global LAST_RESULTS
    LAST_RESULTS = res
    out = np.concatenate([r["out"] for r in res.results], axis=0)
    return out.reshape(B, NTOK, C)



# revision 2
# speedup vs baseline: 4.0084x; 4.0084x over previous
"""Windowed attention (swin-style, 49-token windows, 8 heads) with DynamicPosBias.

v2 design (vs v1 baseline):
- DynamicPosBias MLP + rel-index gather + exp() computed on HOST (tiny,
  replicated); device receives E = exp(rpb) as a [128, 392] f16 table and
  applies it as one elementwise multiply after exp(QK).
- Two windows stacked per 128 SBUF partitions (window A -> partitions 0-63,
  window B -> 64-127, head-dim K=64).  QK and PV matmuls for the two windows
  run as concurrent diagonal PE tiles at tile_position (0,0) / (64,64)
  (auto-derived from base partitions); vector/scalar ops run once per pair
  at ~full 128-partition width.
- All HBM tensors are partition-major [NGRP, 128, big-contiguous] so every
  DMA is a plain [128, multi-KB-lines] transfer (128 descriptors), batched
  GRP=8 window-pairs per dma_start.
- Output written f16 (cast to f32 on host), halving output HBM traffic.
- Software-pipelined with LAG=2 pairs between QK and PV stages.
"""

import numpy as np
from contextlib import ExitStack

import concourse.bass as bass
import concourse.mybir as mybir
import concourse.tile as tile
from concourse import bacc
from concourse.bass_utils import run_bass_kernel_spmd

G = 7
NTOK = 49           # tokens per window
H = 8               # heads
HD = 64             # head dim
C = 512
B = 2048
NCORES = 8
W = B // NCORES     # windows per core (256)
NPAIR = W // 2      # window pairs per core (128)
GRP = 8             # window pairs per DMA group
NGRP = NPAIR // GRP
LAG = 2             # software pipeline depth (pairs) between QK and PV
QCOLS = H * NTOK    # 392
VCOLS = H * (HD + 1)  # 520
F32 = mybir.dt.float32
F16 = mybir.dt.float16


def _rel_idx():
    coords = np.stack(np.meshgrid(np.arange(G), np.arange(G), indexing="ij")).reshape(2, -1)
    rel = (coords[:, :, None] - coords[:, None, :]).transpose(1, 2, 0)
    rel = rel.copy()
    rel[:, :, 0] += G - 1
    rel[:, :, 1] += G - 1
    rel[:, :, 0] *= 2 * G - 1
    return rel.sum(-1)  # [i, j] in [0, 169)


def _host_pos_bias(pos_proj_w, pos_proj_b, ln1_g, ln1_b, w1, b1,
                   ln2_g, ln2_b, w2, b2, ln3_g, ln3_b, w3, b3):
    """Full DynamicPosBias MLP on host -> rpb [H, 49, 49] f32."""
    pb = np.arange(1 - G, G, dtype=np.float32)
    biases = np.stack(np.meshgrid(pb, pb, indexing="ij")).reshape(2, -1).T  # [169, 2]

    def ln(x, g, b, eps=1e-5):
        mu = x.mean(-1, keepdims=True)
        var = ((x - mu) ** 2).mean(-1, keepdims=True)
        return (x - mu) / np.sqrt(var + eps) * g + b

    pos = biases @ pos_proj_w + pos_proj_b
    pos = np.maximum(ln(pos, ln1_g, ln1_b), 0.0) @ w1 + b1
    pos = np.maximum(ln(pos, ln2_g, ln2_b), 0.0) @ w2 + b2
    pos = np.maximum(ln(pos, ln3_g, ln3_b), 0.0) @ w3 + b3  # [169, H]
    rpb = pos[_rel_idx()]          # [i, j, H]
    return rpb.transpose(2, 0, 1)  # [H, i, j]


_CACHED_NC = None
LAST_RESULTS = None


def _build_nc():
    global _CACHED_NC
    if _CACHED_NC is not None:
        return _CACHED_NC
    nc = bacc.Bacc(None, target_bir_lowering=False)

    qk_d = nc.dram_tensor("qk", [NGRP, 128, GRP * 2 * QCOLS], F16, kind="ExternalInput")
    v_d = nc.dram_tensor("vaug", [NGRP, 128, GRP * VCOLS], F16, kind="ExternalInput")
    ef_d = nc.dram_tensor("efull", [128, QCOLS], F16, kind="ExternalInput")
    out_d = nc.dram_tensor("out", [NGRP, 128, GRP * C], F16, kind="ExternalOutput")

    with tile.TileContext(nc) as tc, ExitStack() as ctx:
        const = ctx.enter_context(tc.tile_pool(name="const", bufs=1))
        qkpool = ctx.enter_context(tc.tile_pool(name="qkpool", bufs=3))
        vpool = ctx.enter_context(tc.tile_pool(name="vpool", bufs=3))
        opool = ctx.enter_context(tc.tile_pool(name="opool", bufs=2))
        expool = ctx.enter_context(tc.tile_pool(name="expool", bufs=3))
        exbpool = ctx.enter_context(tc.tile_pool(name="exbpool", bufs=3))
        recpool = ctx.enter_context(tc.tile_pool(name="recpool", bufs=3))
        stps = ctx.enter_context(tc.tile_pool(name="stps", bufs=3, space="PSUM"))
        pvps = ctx.enter_context(tc.tile_pool(name="pvps", bufs=2, space="PSUM"))

        ef_t = const.tile([128, QCOLS], F16, tag="ef")
        nc.sync.dma_start(ef_t[:], ef_d[:])

        stash = {}    # pair index -> (st, v_t, o_t, p_in_group, grp)
        o_last = {}   # grp -> o_t tile

        def do_pv(gp):
            st, v_t, o_t, p, g = stash.pop(gp)
            ex = expool.tile([128, QCOLS], F16, tag="ex")
            nc.scalar.activation(ex[:], st[:], mybir.ActivationFunctionType.Exp)
            exb = exbpool.tile([128, QCOLS], F16, tag="exb")
            nc.vector.tensor_tensor(out=exb[:], in0=ex[:], in1=ef_t[:],
                                    op=mybir.AluOpType.mult)
            pv0 = pvps.tile([128, 4 * 65], F32, tag="pv0")
            pv1 = pvps.tile([128, 4 * 65], F32, tag="pv1")
            for h in range(H):
                dst = pv0 if h < 4 else pv1
                m = h % 4
                for b_ in (0, 64):
                    nc.tensor.matmul(
                        out=dst[b_:b_ + NTOK, 65 * m:65 * (m + 1)],
                        lhsT=exb[b_:b_ + NTOK, NTOK * h:NTOK * (h + 1)],
                        rhs=v_t[b_:b_ + NTOK, VCOLS * p + 65 * h:VCOLS * p + 65 * (h + 1)],
                        start=True, stop=True,
                    )
            rec = recpool.tile([128, H], F32, tag="rec")
            nc.vector.reciprocal(
                rec[:, 0:4].rearrange("p (h o) -> p h o", o=1),
                pv0[:].rearrange("p (h c) -> p h c", c=65)[:, :, 64:65],
            )
            nc.vector.reciprocal(
                rec[:, 4:8].rearrange("p (h o) -> p h o", o=1),
                pv1[:].rearrange("p (h c) -> p h c", c=65)[:, :, 64:65],
            )
            for half, pv in ((0, pv0), (1, pv1)):
                nc.vector.tensor_tensor(
                    out=o_t[:, C * p + 256 * half:C * p + 256 * (half + 1)].rearrange(
                        "p (h c) -> p h c", c=HD),
                    in0=pv[:].rearrange("p (h c) -> p h c", c=65)[:, :, 0:HD],
                    in1=rec[:, 4 * half:4 * half + 4]
                    .rearrange("p (h o) -> p h o", o=1)
                    .to_broadcast([128, 4, HD]),
                    op=mybir.AluOpType.mult,
                )
            if p == GRP - 1:
                nc.scalar.dma_start(out_d[g], o_t[:])
                o_last.pop(g, None)

        for g in range(NGRP):
            qk_t = qkpool.tile([128, GRP * 2 * QCOLS], F16, tag="qk")
            nc.sync.dma_start(qk_t[:], qk_d[g])
            v_t = vpool.tile([128, GRP * VCOLS], F16, tag="v")
            nc.sync.dma_start(v_t[:], v_d[g])
            o_t = opool.tile([128, GRP * C], F16, tag="o")
            o_last[g] = o_t
            for p in range(GRP):
                gp = g * GRP + p
                qbase = p * 2 * QCOLS
                kbase = qbase + QCOLS
                st = stps.tile([128, QCOLS], F32, tag="st")
                for h in range(H):
                    for b_ in (0, 64):
                        nc.tensor.matmul(
                            out=st[b_:b_ + NTOK, NTOK * h:NTOK * (h + 1)],
                            lhsT=qk_t[b_:b_ + HD, kbase + NTOK * h:kbase + NTOK * (h + 1)],
                            rhs=qk_t[b_:b_ + HD, qbase + NTOK * h:qbase + NTOK * (h + 1)],
                            start=True, stop=True,
                        )
                stash[gp] = (st, v_t, o_t, p, g)
                if gp >= LAG:
                    do_pv(gp - LAG)
        for gp in sorted(stash.keys()):
            do_pv(gp)

    nc.finalize()
    _CACHED_NC = nc
    return nc


def kernel(q, k, v, pos_proj_w, pos_proj_b, ln1_g, ln1_b, w1, b1,
           ln2_g, ln2_b, w2, b2, ln3_g, ln3_b, w3, b3):
    q = np.asarray(q, dtype=np.float32)
    k = np.asarray(k, dtype=np.float32)
    v = np.asarray(v, dtype=np.float32)

    # host: q pre-scaled by 1/sqrt(hd); per-window transposed layout [Bw, 64, 392]
    qt_all = np.ascontiguousarray(
        (q * np.float32(HD ** -0.5)).astype(np.float16)
        .reshape(B, NTOK, H, HD).transpose(0, 3, 2, 1)).reshape(B, HD, QCOLS)
    kt_all = np.ascontiguousarray(
        k.astype(np.float16).reshape(B, NTOK, H, HD).transpose(0, 3, 2, 1)
    ).reshape(B, HD, QCOLS)

    # v with fused ones column: [Bw, 49, 520]
    va = np.empty((B, NTOK, H, HD + 1), np.float16)
    va[..., :HD] = v.reshape(B, NTOK, H, HD)
    va[..., HD] = 1.0

    # host DynamicPosBias -> E = exp(rpb), duplicated on partition blocks
    rpb = _host_pos_bias(
        np.asarray(pos_proj_w, np.float32), np.asarray(pos_proj_b, np.float32),
        np.asarray(ln1_g, np.float32), np.asarray(ln1_b, np.float32),
        np.asarray(w1, np.float32), np.asarray(b1, np.float32),
        np.asarray(ln2_g, np.float32), np.asarray(ln2_b, np.float32),
        np.asarray(w2, np.float32), np.asarray(b2, np.float32),
        np.asarray(ln3_g, np.float32), np.asarray(ln3_b, np.float32),
        np.asarray(w3, np.float32), np.asarray(b3, np.float32),
    )  # [H, i, j]
    E = np.exp(rpb).transpose(2, 0, 1).reshape(NTOK, QCOLS)  # [j, h*49+i]
    efull = np.zeros((128, QCOLS), np.float16)
    efull[0:NTOK] = E
    efull[64:64 + NTOK] = E

    in_maps = []
    for c in range(NCORES):
        sl = slice(c * W, (c + 1) * W)
        # qk: [NGRP, 128(b*64+d), GRP, 2(q|k), 392]
        qtc = qt_all[sl].reshape(NGRP, GRP, 2, HD, QCOLS)
        ktc = kt_all[sl].reshape(NGRP, GRP, 2, HD, QCOLS)
        qk = np.empty((NGRP, 2, HD, GRP, 2, QCOLS), np.float16)
        qk[:, :, :, :, 0, :] = qtc.transpose(0, 2, 3, 1, 4)
        qk[:, :, :, :, 1, :] = ktc.transpose(0, 2, 3, 1, 4)
        qk = qk.reshape(NGRP, 128, GRP * 2 * QCOLS)

        # v: [NGRP, 128(b*64+j, rows 49-63 zero), GRP, 520]
        vz = np.zeros((NGRP, 2, HD, GRP, VCOLS), np.float16)
        vz[:, :, :NTOK] = va[sl].reshape(NGRP, GRP, 2, NTOK, VCOLS).transpose(0, 2, 3, 1, 4)
        vz = vz.reshape(NGRP, 128, GRP * VCOLS)

        in_maps.append({"qk": qk, "vaug": vz, "efull": efull})

    nc = _build_nc()
    res = run_bass_kernel_spmd(nc, in_maps, core_ids=list(range(NCORES)))
    global LAST_RESULTS
    LAST_RESULTS = res

    out = np.empty((B, NTOK, C), np.float32)
    for c in range(NCORES):
        od = res.results[c]["out"]  # [NGRP, 128, GRP*512] f16
        od = od.reshape(NGRP, 2, HD, GRP, C).transpose(0, 3, 1, 2, 4)
        out[c * W:(c + 1) * W] = od.reshape(W, HD, C)[:, :NTOK, :].astype(np.float32)
    return out
